# revision 5
# baseline (speedup 1.0000x reference)
"""Trainium2 Bass kernel for nn_Concat_26147760898611.

Mean-pool over the word dim of article_concat [256, 2048, 300] and
options_concat [256, 64, 300], concat features -> [256, 600].

Sharding: pure data parallel over batch across 8 NeuronCores
(32 batches per core).

Per-core design (v2 — ragged engine-balanced DMA):
  - Profiling shows SDMA engine 15 on these parts sustains ~25 GB/s vs
    ~30 GB/s for engines 0-14.  With the usual uniform 16-words-per-
    partition layout every engine gets an equal byte share, so engine 15
    (serving SBUF partitions 92-95 and 124-127) finishes ~30 us after
    the rest and owns the critical path.
  - Fix: ragged per-partition word counts per article batch —
    partitions 0..x-1 get 17 words, x..91 and 96..123 get 16, and the
    eight engine-15 partitions (92-95, 124-127) get SLOW_W words, with
    x = 128 - 8*SLOW_W keeping the total at 2048.  Five DMAs per batch
    (one per constant-width partition range), all with per-partition
    contiguous DRAM spans.
  - Data lands in a single persistent SBUF slab [128, SLOTS*17, 300]
    rotated manually through SLOTS column-slots.  The never-DMA'd
    "garbage" columns (beyond each partition's width) are memset to
    zero once at kernel start and never written again, so folds and
    matmuls can safely read full rectangles: zeros contribute nothing.
  - Reduction per batch: two DVE fold levels (17 -> 8 -> 4 plus the
    unpaired col 8), then 5 PE matmuls with a sliding one-hot selector
    routing the partition-dim sum into PSUM row b.  Selector values are
    1/2048 (1/64 for options), so the PSUM result is already the mean
    and the Scalar engine (and its ACT table load) is never used.
  - Options: one uniform [128, 16, 300] tile, partition p holds 16
    words of batch p//4; folded 16->8->4 and reduced with a block
    selector, drained into the output tile early.
  - The last article batch is split into four column-window chunks so
    the post-last-DMA tail (tiny folds + one matmul + copy + store) is
    short; the final window has no engine-15 bytes at all.
  - A burst of dummy matmuls at kernel start warms the PE HAM clock
    gate before real data lands.

Self-contained: hardcodes all shapes; no file reads.
"""

import numpy as np

N_CORES = 8
B = 256  # full batch
BC = B // N_CORES  # 32 batches per core
DIM = 300
AW = 2048  # article words per batch
OW = 64  # options words per batch
P = 128  # SBUF partitions

SLOW_W = 13  # words per slow partition (engine 15: p92-95, p124-127)
X17 = P - 8 * SLOW_W  # partitions 0..X17-1 carry 17 words
W = 17  # slot width (max per-partition words)
SLOTS = 6  # in-flight article slots in the slab

# (p0, p1, width) constant-width partition ranges, in partition order
RANGES = [
    (0, X17, 17),
    (X17, 92, 16),
    (92, 96, SLOW_W),
    (96, 124, 16),
    (124, 128, SLOW_W),
]
assert sum((p1 - p0) * w for p0, p1, w in RANGES) == AW

# tail column windows for the last batch (geometrically shrinking work)
TAIL_WINS = [(0, 8), (8, 12), (12, 14), (14, 17)]

FOLD_BUFS = 3
WARMUP_MMS = 12

_CACHE = {}


def _build_nc():
    import concourse.bacc as bacc
    import concourse.mybir as mybir
    import concourse.tile as tile

    f32 = mybir.dt.float32
    nc = bacc.Bacc("TRN2", target_bir_lowering=False, debug=False)

    art = nc.dram_tensor("article", [BC, AW, DIM], f32, kind="ExternalInput")
    opt = nc.dram_tensor("options", [BC, OW, DIM], f32, kind="ExternalInput")
    sel_a = nc.dram_tensor("sel_a", [P, 2 * BC - 1], f32, kind="ExternalInput")
    sel_o = nc.dram_tensor("sel_o", [P, BC], f32, kind="ExternalInput")
    out = nc.dram_tensor("out", [BC, 2 * DIM], f32, kind="ExternalOutput")

    # [128, 16, 300]: partition p <- 16 consecutive words of batch p//4
    opt_r = opt.ap().rearrange("b (s q) f -> (b s) q f", s=P // BC)

    def art_range_ap(b, p0, p1, w, c0, c1, word_off):
        """DRAM AP for batch b, partitions [p0,p1) of width w, cols [c0,c1).

        word_off is the word offset of partition p0's span within the batch.
        """
        n = p1 - p0
        span = art.ap()[b, word_off : word_off + n * w]  # [n*w, 300]
        return span.rearrange("(p w) f -> p w f", p=n)[:, c0:c1, :]

    with tile.TileContext(nc) as tc:
        with (
            tc.tile_pool(name="const", bufs=1) as cpool,
            tc.tile_pool(name="data", bufs=1) as dpool,
            tc.tile_pool(name="fold", bufs=FOLD_BUFS) as fpool,
            tc.tile_pool(name="psum", bufs=1, space="PSUM") as ppool,
        ):
            # persistent slab: SLOTS column-slots of width W
            slab = cpool.tile([P, SLOTS * W, DIM], f32, tag="slab")
            opt_t = dpool.tile([P, OW // (P // BC), DIM], f32, tag="opt")
            out_t = cpool.tile([BC, 2 * DIM], f32, tag="out")
            sel_a_t = cpool.tile([P, 2 * BC - 1], f32, tag="sel_a")
            sel_o_t = cpool.tile([P, BC], f32, tag="sel_o")

            psum_a = ppool.tile([BC, DIM], f32, tag="psum_a")
            psum_b = ppool.tile([BC, DIM], f32, tag="psum_b")
            psum_w = ppool.tile([BC, 2 * BC - 1], f32, tag="psum_w")

            def slot_cols(s, c0, c1):
                return slab[:, s * W + c0 : s * W + c1, :]

            def emit_batch_dmas(b, s, c0, c1):
                """DMA batch b's cols [c0,c1) into slot s (clipped per range)."""
                off = 0
                for p0, p1, w in RANGES:
                    lo, hi = min(c0, w), min(c1, w)
                    if hi > lo:
                        nc.sync.dma_start(
                            slab[p0:p1, s * W + lo : s * W + hi, :],
                            art_range_ap(b, p0, p1, w, lo, hi, off),
                        )
                    off += (p1 - p0) * w

            # --- zero each slot's cols [SLOW_W, W) across all partitions
            # (covers every never-DMA'd garbage region; the DMAs that follow
            # overwrite the valid parts).  Compute APs must start at a
            # 32-aligned partition, so full-width memsets + ordering do it.
            def zero_slot(s):
                nc.vector.memset(slab[:, s * W + SLOW_W : s * W + W, :], 0.0)

            zero_slot(0)
            # --- first article batch's DMAs lead the queue ---
            emit_batch_dmas(0, 0, 0, W)
            for s in range(1, SLOTS):
                zero_slot(s)
            nc.sync.dma_start(sel_a_t[:], sel_a.ap()[:])
            nc.sync.dma_start(sel_o_t[:], sel_o.ap()[:])
            nc.sync.dma_start(opt_t[:], opt_r)

            # --- PE warmup: flip the HAM clock gate before data lands ---
            for _ in range(WARMUP_MMS):
                nc.tensor.matmul(
                    psum_w[:], sel_o_t[:], sel_a_t[:], start=True, stop=True
                )

            def reduce_slot(s, sel_ap, psum, first, last):
                """Fold slot s (17 cols) and reduce into psum via 5 matmuls."""
                nxt = fpool.tile([P, 8, DIM], f32, tag="nxt")
                nc.vector.tensor_add(nxt[:], slot_cols(s, 0, 8), slot_cols(s, 9, 17))
                nxt2 = fpool.tile([P, 4, DIM], f32, tag="nxt2")
                nc.vector.tensor_add(nxt2[:], nxt[:, 0:4, :], nxt[:, 4:8, :])
                for j in range(4):
                    nc.tensor.matmul(
                        psum[:], sel_ap, nxt2[:, j, :],
                        start=(first and j == 0), stop=False,
                    )
                nc.tensor.matmul(
                    psum[:], sel_ap, slot_cols(s, 8, 9)[:, 0, :],
                    start=False, stop=last,
                )

            def sel_for(b):
                return sel_a_t[:, BC - 1 - b : 2 * BC - 1 - b]

            # batch 0 reduce, then options, then the remaining batches
            reduce_slot(0, sel_for(0), psum_a, True, False)

            # options: fold 16 -> 8 -> 4, then 4 block-selector matmuls
            onx = fpool.tile([P, 8, DIM], f32, tag="nxt")
            nc.vector.tensor_add(onx[:], opt_t[:, 0:8, :], opt_t[:, 8:16, :])
            onx2 = fpool.tile([P, 4, DIM], f32, tag="nxt2")
            nc.vector.tensor_add(onx2[:], onx[:, 0:4, :], onx[:, 4:8, :])
            for j in range(4):
                nc.tensor.matmul(
                    psum_b[:], sel_o_t[:], onx2[:, j, :],
                    start=(j == 0), stop=(j == 3),
                )
            nc.vector.tensor_copy(out_t[:, DIM : 2 * DIM], psum_b[:])

            for b in range(1, BC - 1):
                s = b % SLOTS
                emit_batch_dmas(b, s, 0, W)
                reduce_slot(s, sel_for(b), psum_a, False, False)

            # --- last batch in column windows; final window avoids engine 15
            b = BC - 1
            s = b % SLOTS
            sel_last = sel_for(b)
            for wi, (c0, c1) in enumerate(TAIL_WINS):
                emit_batch_dmas(b, s, c0, c1)
                is_last_win = wi == len(TAIL_WINS) - 1
                nw = c1 - c0
                if nw == 8:
                    t4 = fpool.tile([P, 4, DIM], f32, tag="nxt2")
                    nc.vector.tensor_add(
                        t4[:], slot_cols(s, c0, c0 + 4), slot_cols(s, c0 + 4, c0 + 8)
                    )
                    t2 = fpool.tile([P, 2, DIM], f32, tag="t2", bufs=2)
                    nc.vector.tensor_add(t2[:], t4[:, 0:2, :], t4[:, 2:4, :])
                    cols = [t2[:, 0, :], t2[:, 1, :]]
                elif nw == 4:
                    t2 = fpool.tile([P, 2, DIM], f32, tag="t2", bufs=2)
                    nc.vector.tensor_add(
                        t2[:], slot_cols(s, c0, c0 + 2), slot_cols(s, c0 + 2, c0 + 4)
                    )
                    cols = [t2[:, 0, :], t2[:, 1, :]]
                elif nw == 2:
                    t1 = fpool.tile([P, 1, DIM], f32, tag="t1", bufs=2)
                    nc.vector.tensor_add(
                        t1[:], slot_cols(s, c0, c0 + 1), slot_cols(s, c0 + 1, c0 + 2)
                    )
                    cols = [t1[:, 0, :]]
                else:  # nw == 3
                    t1 = fpool.tile([P, 1, DIM], f32, tag="t1", bufs=2)
                    nc.vector.tensor_add(
                        t1[:], slot_cols(s, c0, c0 + 1), slot_cols(s, c0 + 1, c0 + 2)
                    )
                    t1b = fpool.tile([P, 1, DIM], f32, tag="t1", bufs=2)
                    nc.vector.tensor_add(t1b[:], t1[:], slot_cols(s, c0 + 2, c0 + 3))
                    cols = [t1b[:, 0, :]]
                for j, col in enumerate(cols):
                    nc.tensor.matmul(
                        psum_a[:], sel_last, col,
                        start=False, stop=(is_last_win and j == len(cols) - 1),
                    )

            nc.vector.tensor_copy(out_t[:, 0:DIM], psum_a[:])
            nc.sync.dma_start(out.ap()[:], out_t[:])

    nc.compile()
    return nc


def get_nc():
    if "nc" not in _CACHE:
        _CACHE["nc"] = _build_nc()
    return _CACHE["nc"]


def _sel_arrays():
    # selector values carry the mean scaling (exact powers of two)
    sel_a = np.zeros((P, 2 * BC - 1), np.float32)
    sel_a[:, BC - 1] = 1.0 / AW
    sel_o = np.zeros((P, BC), np.float32)
    sel_o[np.arange(P), np.arange(P) // (P // BC)] = 1.0 / OW
    return sel_a, sel_o


def make_in_maps(article, options):
    article = np.ascontiguousarray(np.asarray(article, dtype=np.float32))
    options = np.ascontiguousarray(np.asarray(options, dtype=np.float32))
    assert article.shape == (B, AW, DIM), article.shape
    assert options.shape == (B, OW, DIM), options.shape
    sel_a, sel_o = _sel_arrays()
    return [
        {
            "article": article[i * BC : (i + 1) * BC],
            "options": options[i * BC : (i + 1) * BC],
            "sel_a": sel_a,
            "sel_o": sel_o,
        }
        for i in range(N_CORES)
    ]


def run_sharded(article, options, **spmd_kwargs):
    from concourse.bass_utils import run_bass_kernel_spmd

    nc = get_nc()
    in_maps = make_in_maps(article, options)
    res = run_bass_kernel_spmd(nc, in_maps, list(range(N_CORES)), **spmd_kwargs)
    full = np.concatenate(
        [res.results[i]["out"] for i in range(N_CORES)], axis=0
    ).astype(np.float32)
    return full, res


def kernel(article_concat, options_concat):
    full, _ = run_sharded(article_concat, options_concat)
    return full


# revision 8
# speedup vs baseline: 2.4694x; 2.4694x over previous
"""Trainium2 Bass kernel for nn_Concat_26147760898611.

Mean-pool over the word dim of article_concat [256, 2048, 300] and
options_concat [256, 64, 300], concat features -> [256, 600].

Sharding: pure data parallel over batch across 8 NeuronCores
(32 batches per core).

Per-core design (v3 — descriptor-deal-aware engine balancing):
  - Profiling shows SDMA engine 15 sustains ~22.4 GB/s vs ~26.1 GB/s
    for engines 0-14, so with uniform layouts engine 15 finishes ~30 us
    after the rest and owns the critical path.
  - HWDGE descriptor dealing (measured): a DMA with n descriptors goes
    to k = (largest divisor of n that is <= 16) engines, starting at
    engine 0, n/k consecutive descriptors each.  A [128, w, 300] DMA
    therefore gives partitions 120-127 to engine 15; a [120, w, 300]
    DMA (120 = 15*8) engages engines 0-14 only; an [8, w, 300] DMA
    engages engines 0-7.
  - Per article batch (2048 words): DMA A [128, 14, 300] covers words
    p*14 for every partition; DMA B [120, 2, 300] covers 2 more words
    for partitions 0-119 (engine 15 skipped); DMA C [8, 2, 300] gives
    the last 16 words to partitions 0-7.  Engine bytes/batch: 156 KB
    (e0-7) / 153.6 KB (e8-14) / 134.4 KB (e15) — matching the measured
    ~0.86 speed ratio, so all engines finish together.
  - Data lands in a persistent SBUF slab [128, SLOTS*18, 300] rotated
    through SLOTS column-slots.  Cols [14,18) of each slot are memset
    to zero before the slot's first use; DMAs overwrite the valid parts
    and the never-written garbage zones stay zero forever, so folds and
    matmuls read full rectangles safely (zeros add nothing).
  - Reduction per batch: DVE folds 18 -> 9 -> (4 + unpaired col 8),
    then 5 PE matmuls with a sliding one-hot selector routing the
    partition-dim sum into PSUM row b.  Selector values are 1/2048
    (1/64 for options) so PSUM holds the mean directly and the Scalar
    engine (and its ACT table load) is never used.
  - Options gets the same ragged treatment (its 2048 words = exactly
    one article batch): partition p holds words of batch p//4; the
    8 leftover 2-word chunks from partitions 120-127 ride on partitions
    0-7 cols 16-18 with a second selector.
  - The last article batch is processed in four shrinking column
    windows so the post-last-DMA tail is tiny.
  - A burst of dummy matmuls at kernel start warms the PE HAM clock
    gate before real data lands.

Self-contained: hardcodes all shapes; no file reads.
"""

import numpy as np

N_CORES = 8
B = 256  # full batch
BC = B // N_CORES  # 32 batches per core
DIM = 300
AW = 2048  # article words per batch
OW = 64  # options words per batch
P = 128  # SBUF partitions

AWIDE = 14  # DMA A width (all 128 partitions)
BWIDE = 2  # DMA B width (partitions 0-119; engine 15 skipped)
CWIDE = 2  # DMA C width (partitions 0-7)
W = AWIDE + BWIDE + CWIDE  # 18: slot width
SLOTS = 5  # in-flight article slots in the slab
assert 128 * AWIDE + 120 * BWIDE + 8 * CWIDE == AW

# DMA plan per article batch: (p0, p1, col0, col1, dram word offset)
PLAN = [
    (0, P, 0, AWIDE, 0),
    (0, 120, AWIDE, AWIDE + BWIDE, P * AWIDE),
    (0, 8, AWIDE + BWIDE, W, P * AWIDE + 120 * BWIDE),
]

# tail column windows for the last batch (shrinking work)
TAIL_WINS = [(0, 8), (8, 12), (12, 15), (15, 18)]

FOLD_BUFS = 3
WARMUP_MMS = 12

_CACHE = {}


def _build_nc():
    import concourse.bacc as bacc
    import concourse.mybir as mybir
    import concourse.tile as tile

    f32 = mybir.dt.float32
    nc = bacc.Bacc("TRN2", target_bir_lowering=False, debug=False)

    art = nc.dram_tensor("article", [BC, AW, DIM], f32, kind="ExternalInput")
    opt = nc.dram_tensor("options", [BC, OW, DIM], f32, kind="ExternalInput")
    sel_a = nc.dram_tensor("sel_a", [P, 2 * BC - 1], f32, kind="ExternalInput")
    sel_o = nc.dram_tensor("sel_o", [P, BC + 2], f32, kind="ExternalInput")
    out = nc.dram_tensor("out", [BC, 2 * DIM], f32, kind="ExternalOutput")

    # options stream viewed as [128, 16, 300]: partition p <- batch p//4
    opt_r = opt.ap().rearrange("b (s q) f -> (b s) q f", s=P // BC)

    with tile.TileContext(nc) as tc:
        with (
            tc.tile_pool(name="const", bufs=1) as cpool,
            tc.tile_pool(name="fold", bufs=FOLD_BUFS) as fpool,
            tc.tile_pool(name="psum", bufs=1, space="PSUM") as ppool,
        ):
            # persistent slab: SLOTS column-slots of width W
            slab = cpool.tile([P, SLOTS * W, DIM], f32, tag="slab")
            opt_t = cpool.tile([P, W, DIM], f32, tag="opt")
            out_t = cpool.tile([BC, 2 * DIM], f32, tag="out")
            sel_a_t = cpool.tile([P, 2 * BC - 1], f32, tag="sel_a")
            sel_o_t = cpool.tile([P, BC + 2], f32, tag="sel_o")

            psum_a = ppool.tile([BC, DIM], f32, tag="psum_a")
            psum_b = ppool.tile([BC, DIM], f32, tag="psum_b")
            psum_w = ppool.tile([BC, 2 * BC - 1], f32, tag="psum_w")

            def slot_cols(s, c0, c1):
                return slab[:, s * W + c0 : s * W + c1, :]

            def emit_batch_dmas(b, s, c0, c1):
                """DMA batch b's cols [c0,c1) into slot s (clipped per plan)."""
                for p0, p1, C0, C1, off in PLAN:
                    lo, hi = max(c0, C0), min(c1, C1)
                    if hi > lo:
                        n = p1 - p0
                        src = art.ap()[b, off : off + n * (C1 - C0)]
                        src = src.rearrange("(p w) f -> p w f", p=n)
                        nc.sync.dma_start(
                            slab[p0:p1, s * W + lo : s * W + hi, :],
                            src[:, lo - C0 : hi - C0, :],
                        )

            # zero cols [AWIDE, W) of a slot; following DMAs overwrite the
            # valid parts and the garbage zones stay zero forever
            def zero_slot(s):
                nc.vector.memset(slab[:, s * W + AWIDE : s * W + W, :], 0.0)

            zero_slot(0)
            # --- first article batch's DMAs lead the queue ---
            emit_batch_dmas(0, 0, 0, W)
            for s in range(1, SLOTS):
                zero_slot(s)
            nc.sync.dma_start(sel_a_t[:], sel_a.ap()[:])
            nc.sync.dma_start(sel_o_t[:], sel_o.ap()[:])

            # --- options: same ragged split (2048 words total) ---
            nc.vector.memset(opt_t[:, AWIDE:W, :], 0.0)
            opt_flat = opt.ap().rearrange("b w f -> (b w) f")
            o_a = opt_flat[0 : P * 16].rearrange("(p w) f -> p w f", p=P)
            nc.sync.dma_start(opt_t[:, 0:AWIDE, :], o_a[:, 0:AWIDE, :])
            nc.sync.dma_start(
                opt_t[0:120, AWIDE : AWIDE + BWIDE, :], o_a[0:120, AWIDE:16, :]
            )
            nc.sync.dma_start(
                opt_t[0:8, AWIDE + BWIDE : W, :], o_a[120:128, AWIDE:16, :]
            )

            # --- PE warmup: flip the HAM clock gate before data lands ---
            for _ in range(WARMUP_MMS):
                nc.tensor.matmul(
                    psum_w[:], sel_o_t[:, 0:BC], sel_a_t[:],
                    start=True, stop=True,
                )

            def reduce_slot(s, sel_ap, psum, first, last):
                """Fold slot s (18 cols) and reduce into psum via 5 matmuls."""
                nxt = fpool.tile([P, 9, DIM], f32, tag="nxt")
                nc.vector.tensor_add(nxt[:], slot_cols(s, 0, 9), slot_cols(s, 9, 18))
                nxt2 = fpool.tile([P, 4, DIM], f32, tag="nxt2")
                nc.vector.tensor_add(nxt2[:], nxt[:, 0:4, :], nxt[:, 4:8, :])
                for j in range(4):
                    nc.tensor.matmul(
                        psum[:], sel_ap, nxt2[:, j, :],
                        start=(first and j == 0), stop=False,
                    )
                nc.tensor.matmul(
                    psum[:], sel_ap, nxt[:, 8, :], start=False, stop=last
                )

            def sel_for(b):
                return sel_a_t[:, BC - 1 - b : 2 * BC - 1 - b]

            # batch 0 reduce, then options, then the remaining batches
            reduce_slot(0, sel_for(0), psum_a, True, False)

            # options: cols 0-16 via main selector, cols 16-18 via aux
            onx = fpool.tile([P, 8, DIM], f32, tag="nxt")
            nc.vector.tensor_add(onx[:], opt_t[:, 0:8, :], opt_t[:, 8:16, :])
            onx2 = fpool.tile([P, 4, DIM], f32, tag="nxt2")
            nc.vector.tensor_add(onx2[:], onx[:, 0:4, :], onx[:, 4:8, :])
            for j in range(4):
                nc.tensor.matmul(
                    psum_b[:], sel_o_t[:, 0:BC], onx2[:, j, :],
                    start=(j == 0), stop=False,
                )
            oc = fpool.tile([P, 1, DIM], f32, tag="t1", bufs=2)
            nc.vector.tensor_add(oc[:], opt_t[:, 16:17, :], opt_t[:, 17:18, :])
            # window [2, BC+2): position 30/31 = aux cols BC/BC+1 -> rows 30/31;
            # main-selector cols 2..BC-1 in the window only see zero rows of oc
            nc.tensor.matmul(
                psum_b[:], sel_o_t[:, 2 : BC + 2],
                oc[:, 0, :], start=False, stop=True,
            )
            nc.vector.tensor_copy(out_t[:, DIM : 2 * DIM], psum_b[:])

            for b in range(1, BC - 1):
                s = b % SLOTS
                emit_batch_dmas(b, s, 0, W)
                reduce_slot(s, sel_for(b), psum_a, False, False)

            # --- last batch in shrinking column windows ---
            b = BC - 1
            s = b % SLOTS
            sel_last = sel_for(b)
            for wi, (c0, c1) in enumerate(TAIL_WINS):
                emit_batch_dmas(b, s, c0, c1)
                is_last_win = wi == len(TAIL_WINS) - 1
                nw = c1 - c0
                if nw == 8:
                    t4 = fpool.tile([P, 4, DIM], f32, tag="nxt2")
                    nc.vector.tensor_add(
                        t4[:], slot_cols(s, c0, c0 + 4), slot_cols(s, c0 + 4, c0 + 8)
                    )
                    t2 = fpool.tile([P, 2, DIM], f32, tag="t2", bufs=2)
                    nc.vector.tensor_add(t2[:], t4[:, 0:2, :], t4[:, 2:4, :])
                    cols = [t2[:, 0, :], t2[:, 1, :]]
                elif nw == 4:
                    t2 = fpool.tile([P, 2, DIM], f32, tag="t2", bufs=2)
                    nc.vector.tensor_add(
                        t2[:], slot_cols(s, c0, c0 + 2), slot_cols(s, c0 + 2, c0 + 4)
                    )
                    cols = [t2[:, 0, :], t2[:, 1, :]]
                else:  # nw == 3
                    t1 = fpool.tile([P, 1, DIM], f32, tag="t1", bufs=2)
                    nc.vector.tensor_add(
                        t1[:], slot_cols(s, c0, c0 + 1), slot_cols(s, c0 + 1, c0 + 2)
                    )
                    t1b = fpool.tile([P, 1, DIM], f32, tag="t1", bufs=2)
                    nc.vector.tensor_add(t1b[:], t1[:], slot_cols(s, c0 + 2, c0 + 3))
                    cols = [t1b[:, 0, :]]
                for j, col in enumerate(cols):
                    nc.tensor.matmul(
                        psum_a[:], sel_last, col,
                        start=False, stop=(is_last_win and j == len(cols) - 1),
                    )

            nc.vector.tensor_copy(out_t[:, 0:DIM], psum_a[:])
            nc.sync.dma_start(out.ap()[:], out_t[:])

    nc.compile()
    return nc


def get_nc():
    if "nc" not in _CACHE:
        _CACHE["nc"] = _build_nc()
    return _CACHE["nc"]


def _sel_arrays():
    # selector values carry the mean scaling (exact powers of two)
    sel_a = np.zeros((P, 2 * BC - 1), np.float32)
    sel_a[:, BC - 1] = 1.0 / AW
    # cols 0..BC-1: partition p -> batch p//4 (options main layout)
    # cols BC..BC+2: aux selector for the 8 leftover chunks riding on
    # partitions 0-7 (partition q holds batch 30 + q//4's words)
    sel_o = np.zeros((P, BC + 2), np.float32)
    sel_o[np.arange(P), np.arange(P) // (P // BC)] = 1.0 / OW
    for q in range(8):
        sel_o[q, BC + q // 4] = 1.0 / OW
    return sel_a, sel_o


def make_in_maps(article, options):
    article = np.ascontiguousarray(np.asarray(article, dtype=np.float32))
    options = np.ascontiguousarray(np.asarray(options, dtype=np.float32))
    assert article.shape == (B, AW, DIM), article.shape
    assert options.shape == (B, OW, DIM), options.shape
    sel_a, sel_o = _sel_arrays()
    return [
        {
            "article": article[i * BC : (i + 1) * BC],
            "options": options[i * BC : (i + 1) * BC],
            "sel_a": sel_a,
            "sel_o": sel_o,
        }
        for i in range(N_CORES)
    ]


def run_sharded(article, options, **spmd_kwargs):
    from concourse.bass_utils import run_bass_kernel_spmd

    nc = get_nc()
    in_maps = make_in_maps(article, options)
    res = run_bass_kernel_spmd(nc, in_maps, list(range(N_CORES)), **spmd_kwargs)
    full = np.concatenate(
        [res.results[i]["out"] for i in range(N_CORES)], axis=0
    ).astype(np.float32)
    return full, res


def kernel(article_concat, options_concat):
    full, _ = run_sharded(article_concat, options_concat)
    return full


# revision 10
# speedup vs baseline: 2.7784x; 1.1251x over previous
"""Trainium2 Bass kernel for nn_Concat_26147760898611.

Mean-pool over the word dim of article_concat [256, 2048, 300] and
options_concat [256, 64, 300], concat features -> [256, 600].

Sharding: pure data parallel over batch across 8 NeuronCores
(32 batches per core).

Per-core design (v4 — descriptor-deal engine balancing, fat descriptors):
  - Profiling shows SDMA engine 15 sustains ~22.4 GB/s vs ~26.1 GB/s for
    engines 0-14, so with a uniform layout engine 15 finishes ~30 us
    after the rest and owns the critical path.
  - HWDGE descriptor dealing (measured): a DMA with n descriptors is
    dealt to k = (largest divisor of n <= 16) engines, starting at
    engine 0, n/k consecutive descriptors each.  The SBUF partition a
    descriptor targets is irrelevant to which engine moves it.
  - So: all DMAs keep fat 19.2 KB per-partition descriptors, and
    SKIP_BATCHES article batches are loaded by a [120, 16, 300] DMA
    (120 = 15*8 -> engines 0-14 only) plus an [8, 16, 300] DMA
    (engines 0-7), which sheds exactly those batches' bytes from
    engine 15.  With 4 skip batches: engine 15 moves 28/33 units
    (199 us at 22.4 GB/s), engines 0-7 33 units + 4/8 extra descriptors
    (~197 us at 26.1 GB/s) — all engines finish together.
  - Each article batch [2048, 300] lands in an SBUF tile
    [128 partitions, 16 words, 300]; partition p holds 16 consecutive
    words (19.2 KB contiguous per partition).  The word axis folds
    16 -> 8 -> 4 on the VectorEngine (fp32-exact adds); 4 TensorEngine
    matmuls with a sliding one-hot selector reduce across partitions
    into PSUM row b.  Selector values are 1/2048 (1/64 for options), so
    PSUM holds the mean directly and the Scalar engine (and its ACT
    table preamble load) is never used; DVE copies PSUM -> out tile.
  - Options: partition p holds 16 consecutive words of batch p//4, one
    block-selector reduction, drained into the output tile early.
  - The last batch is split into shrinking chunks so the post-last-DMA
    tail (fold + matmul + copy + store) is short.
  - A burst of dummy matmuls at kernel start warms the PE HAM clock
    gate (1.2 -> 2.4 GHz) before real data lands.

Self-contained: hardcodes all shapes; no file reads.
"""

import numpy as np

N_CORES = 8
B = 256  # full batch
BC = B // N_CORES  # 32 batches per core
DIM = 300
AW = 2048  # article words per batch
OW = 64  # options words per batch
P = 128  # SBUF partitions
AWP = AW // P  # 16 article words per partition

SKIP_BATCHES = (6, 13, 20, 27)  # batches whose DMAs bypass engine 15
TAIL_CHUNKS = [8, 4, 2, 1, 1]  # geometric split of the final batch
DATA_BUFS = 6
FOLD_BUFS = 3
WARMUP_MMS = 12

_CACHE = {}


def _build_nc():
    import concourse.bacc as bacc
    import concourse.mybir as mybir
    import concourse.tile as tile

    f32 = mybir.dt.float32
    nc = bacc.Bacc("TRN2", target_bir_lowering=False, debug=False)

    art = nc.dram_tensor("article", [BC, AW, DIM], f32, kind="ExternalInput")
    opt = nc.dram_tensor("options", [BC, OW, DIM], f32, kind="ExternalInput")
    sel_a = nc.dram_tensor("sel_a", [P, 2 * BC - 1], f32, kind="ExternalInput")
    sel_o = nc.dram_tensor("sel_o", [P, BC], f32, kind="ExternalInput")
    out = nc.dram_tensor("out", [BC, 2 * DIM], f32, kind="ExternalOutput")

    # [BC, 128, 16, 300]: partition p <- words p*16 .. p*16+15 (contiguous)
    art_r = art.ap().rearrange("b (p w) f -> b p w f", p=P)
    # per-partition word view of the last batch: [128, 16, 300]
    art_last = art.ap()[BC - 1].rearrange("(p w) f -> p w f", p=P)
    # [128, 16, 300]: partition p <- 16 consecutive words of batch p//4
    opt_r = opt.ap().rearrange("b (s q) f -> (b s) q f", s=P // BC)

    with tile.TileContext(nc) as tc:
        with (
            tc.tile_pool(name="const", bufs=1) as cpool,
            tc.tile_pool(name="data", bufs=DATA_BUFS) as dpool,
            tc.tile_pool(name="fold", bufs=FOLD_BUFS) as fpool,
            tc.tile_pool(name="outp", bufs=1) as opool,
            tc.tile_pool(name="psum", bufs=1, space="PSUM") as ppool,
        ):
            sel_a_t = cpool.tile([P, 2 * BC - 1], f32, tag="sel_a")
            sel_o_t = cpool.tile([P, BC], f32, tag="sel_o")
            out_t = opool.tile([BC, 2 * DIM], f32, tag="out")

            psum_a = ppool.tile([BC, DIM], f32, tag="psum_a")
            psum_b = ppool.tile([BC, DIM], f32, tag="psum_b")
            psum_w = ppool.tile([BC, 2 * BC - 1], f32, tag="psum_w")

            def load_batch(b):
                """DMA article batch b; skip batches bypass engine 15."""
                t = dpool.tile([P, AWP, DIM], f32, tag="data")
                if b in SKIP_BATCHES:
                    # 120 descriptors -> engines 0-14; 8 -> engines 0-7
                    nc.sync.dma_start(t[0:120], art_r[b, 0:120])
                    nc.sync.dma_start(t[120:P], art_r[b, 120:P])
                else:
                    nc.sync.dma_start(t[:], art_r[b])
                return t

            def reduce_tile(t, nch, sel_ap, psum, first, last):
                """Fold nch cols twice on DVE, then matmul-reduce into psum."""
                cur, n = t, nch
                for lvl in range(2):
                    if n == 1:
                        break
                    n //= 2
                    nxt = fpool.tile([P, n, DIM], f32, tag=f"fold{lvl}_{nch}")
                    nc.vector.tensor_add(nxt[:], cur[:, 0:n, :], cur[:, n : 2 * n, :])
                    cur = nxt
                for j in range(n):
                    nc.tensor.matmul(
                        psum[:], sel_ap, cur[:, j, :],
                        start=(first and j == 0), stop=(last and j == n - 1),
                    )

            def sel_for(b):
                return sel_a_t[:, BC - 1 - b : 2 * BC - 1 - b]

            # first article batch's DMA leads the queue
            t0 = load_batch(0)
            nc.sync.dma_start(sel_a_t[:], sel_a.ap()[:])
            nc.sync.dma_start(sel_o_t[:], sel_o.ap()[:])
            opt_t = dpool.tile([P, AWP, DIM], f32, tag="data")
            nc.sync.dma_start(opt_t[:], opt_r)

            # PE warmup: flip the HAM clock gate to 2.4 GHz early.
            for _ in range(WARMUP_MMS):
                nc.tensor.matmul(
                    psum_w[:], sel_o_t[:], sel_a_t[:], start=True, stop=True
                )

            reduce_tile(t0, AWP, sel_for(0), psum_a, True, False)

            # options; drain its psum into the output tile early
            reduce_tile(opt_t, AWP, sel_o_t[:], psum_b, True, True)
            nc.vector.tensor_copy(out_t[:, DIM : 2 * DIM], psum_b[:])

            for b in range(1, BC - 1):
                t = load_batch(b)
                reduce_tile(t, AWP, sel_for(b), psum_a, False, False)

            # final batch in geometrically shrinking chunks -> the very
            # last DMA is tiny and its fold+matmul tail is short
            sel_last = sel_for(BC - 1)
            assert sum(TAIL_CHUNKS) == AWP
            w0 = 0
            for i, nch in enumerate(TAIL_CHUNKS):
                t = dpool.tile([P, nch, DIM], f32, tag="data")
                nc.sync.dma_start(t[:], art_last[:, w0 : w0 + nch, :])
                reduce_tile(
                    t, nch, sel_last, psum_a, False, i == len(TAIL_CHUNKS) - 1
                )
                w0 += nch

            nc.vector.tensor_copy(out_t[:, 0:DIM], psum_a[:])
            nc.sync.dma_start(out.ap()[:], out_t[:])

    nc.compile()
    return nc


def get_nc():
    if "nc" not in _CACHE:
        _CACHE["nc"] = _build_nc()
    return _CACHE["nc"]


def _sel_arrays():
    # selector values carry the mean scaling (exact powers of two)
    sel_a = np.zeros((P, 2 * BC - 1), np.float32)
    sel_a[:, BC - 1] = 1.0 / AW
    sel_o = np.zeros((P, BC), np.float32)
    sel_o[np.arange(P), np.arange(P) // (P // BC)] = 1.0 / OW
    return sel_a, sel_o


def make_in_maps(article, options):
    article = np.ascontiguousarray(np.asarray(article, dtype=np.float32))
    options = np.ascontiguousarray(np.asarray(options, dtype=np.float32))
    assert article.shape == (B, AW, DIM), article.shape
    assert options.shape == (B, OW, DIM), options.shape
    sel_a, sel_o = _sel_arrays()
    return [
        {
            "article": article[i * BC : (i + 1) * BC],
            "options": options[i * BC : (i + 1) * BC],
            "sel_a": sel_a,
            "sel_o": sel_o,
        }
        for i in range(N_CORES)
    ]


def run_sharded(article, options, **spmd_kwargs):
    from concourse.bass_utils import run_bass_kernel_spmd

    nc = get_nc()
    in_maps = make_in_maps(article, options)
    res = run_bass_kernel_spmd(nc, in_maps, list(range(N_CORES)), **spmd_kwargs)
    full = np.concatenate(
        [res.results[i]["out"] for i in range(N_CORES)], axis=0
    ).astype(np.float32)
    return full, res


def kernel(article_concat, options_concat):
    full, _ = run_sharded(article_concat, options_concat)
    return full
